# revision 3
# baseline (speedup 1.0000x reference)
"""CrossAttention (3x 3D-conv projections + channel attention + residual)
on 8 Trainium2 NeuronCores, data-parallel over batch (2 batches/core).

v3 vs baseline:
  - all conv/attention matmul operands in fp16 (PE rate identical to
    fp32r at free-dim 512, but half the HBM traffic for the 170MB/core
    of weights; fp32 PSUM accumulation keeps rel err ~6e-4)
  - q/k/vT stay resident in SBUF between conv and attention (no DRAM
    scratch spill/reload)
  - scores+softmax for both batches are emitted between the k and v
    convs so the DVE/Act softmax work hides under v-conv PE time
  - weights regrouped host-side to [ICH, oh, 3, 128, 9, 512] so each
    weight DMA moves 9 taps as one fully-contiguous 9KB/partition block
    (144 weight DMAs instead of 1296; less descriptor-generation and
    better HBM burst efficiency)
  - xres is streamed per output tile instead of held resident
"""
import sys

sys.path.insert(0, '/opt/trn_rl_repo')

import ml_dtypes
import numpy as np

from concourse import bacc, mybir, masks
from concourse.tile import TileContext
from concourse.bass_utils import run_bass_kernel_spmd

F32 = mybir.dt.float32
F16 = mybir.dt.float16
F8 = mybir.dt.float8e4
WS = 512.0                 # fp8 weight pre-scale (descaled at PSUM evac)

B, C, N = 16, 1024, 512
NCORES = 8
BPC = B // NCORES          # batches per core
ICH = C // 128             # input/output channel chunks of 128
PAD = 1000                 # 10*10*10 padded volume

_CACHED_NC = None
LAST_RESULTS = None


def _taps():
    for t in range(27):
        yield t, t // 9, (t // 3) % 3, t % 3


def _build():
    nc = bacc.Bacc("TRN2", target_bir_lowering=False, debug=False)

    xp = nc.dram_tensor("xp", [BPC, C, PAD], F16, kind="ExternalInput")
    yp = nc.dram_tensor("yp", [BPC, C, PAD], F16, kind="ExternalInput")
    xres = nc.dram_tensor("xres", [BPC, C, N], F32, kind="ExternalInput")
    WSHAPE = [ICH, 2, 3, 128, 9, 512]
    wqp = nc.dram_tensor("wqp", WSHAPE, F16, kind="ExternalInput")
    wkp = nc.dram_tensor("wkp", WSHAPE, F16, kind="ExternalInput")
    # v projection in fp8 (DoubleRow): weights [t, icp, p, j, oc],
    # activation tap-slabs [b, t, icp, p, j, n]
    wv8 = nc.dram_tensor("wv8", [27, 4, 128, 2, 1024], F8,
                         kind="ExternalInput")
    y8sl = nc.dram_tensor("y8sl", [BPC, 27, 4, 128, 2, 512], F8,
                          kind="ExternalInput")
    bqp = nc.dram_tensor("bqp", [128, ICH], F32, kind="ExternalInput")
    bkp = nc.dram_tensor("bkp", [128, ICH], F32, kind="ExternalInput")
    out = nc.dram_tensor("out", [BPC, C, N], F32, kind="ExternalOutput")

    with TileContext(nc) as tc:
        with tc.tile_pool(name="const", bufs=1) as cpool, \
             tc.tile_pool(name="psum", bufs=1, space="PSUM") as psp:

            ident = cpool.tile([128, 128], F32, tag="ident")
            masks.make_identity(nc, ident[:])
            bq_t = cpool.tile([128, ICH], F32, tag="bq_t")
            nc.sync.dma_start(bq_t[:], bqp[:])
            bk_t = cpool.tile([128, ICH], F32, tag="bk_t")
            nc.sync.dma_start(bk_t[:], bkp[:])

            def psum_tile(i):
                return psp.tile([128, 512], F32, tag=f"ps{i}", name=f"ps{i}")

            # persistent projection outputs (SBUF-resident, fp16)
            q16 = cpool.tile([128, BPC, ICH, N], F16, tag="q16")
            k16 = cpool.tile([128, BPC, ICH, N], F16, tag="k16")
            vT16 = cpool.tile([128, BPC, 4, C], F16, tag="vT16")

            with tc.tile_pool(name="acts", bufs=1) as apool, \
                 tc.tile_pool(name="wts", bufs=3) as wpool, \
                 tc.tile_pool(name="ev", bufs=4) as evpool:

                # padded activations: [128, ic 8, 10, 10, 10] fp16
                def load_pad(src, name):
                    tiles = []
                    for b in range(BPC):
                        t = apool.tile([128, ICH, 10, 10, 10], F16,
                                       tag=f"{name}{b}", name=f"{name}{b}")
                        nc.sync.dma_start(
                            t[:].rearrange("p i a b c -> p i (a b c)"),
                            src[b].rearrange("(i p) n -> p i n", p=128))
                        tiles.append(t)
                    return tiles

                xpad = load_pad(xp, "xpad")
                ypad = load_pad(yp, "ypad")

                def conv(pads, w_dram, bias_t, transposed, dst16=None):
                    """27x8-step PSUM accumulation per oh half; evacuate
                    with bias into fp16 SBUF (q/k) or TensorE-transpose
                    into vT16 (v)."""
                    for oh in range(2):
                        pq = [psum_tile(i) for i in range(8)]
                        for ic in range(ICH):
                            for tg in range(3):
                                wt = wpool.tile([128, 9, 512], F16, tag="wt",
                                                name="wt")
                                nc.sync.dma_start(wt[:], w_dram[ic, oh, tg])
                                for t9 in range(9):
                                    t = tg * 9 + t9
                                    kd, kh, kw = t // 9, (t // 3) % 3, t % 3
                                    first = (ic == 0 and t == 0)
                                    last = (ic == ICH - 1 and t == 26)
                                    for ol in range(4):
                                        lhsT = wt[:, t9,
                                                  ol * 128:(ol + 1) * 128]
                                        for b in range(BPC):
                                            slab = pads[b][:, ic, kd:kd + 8,
                                                           kh:kh + 8,
                                                           kw:kw + 8]
                                            nc.tensor.matmul(
                                                pq[ol * BPC + b][:], lhsT,
                                                slab, start=first, stop=last)
                        for ol in range(4):
                            oc = oh * 4 + ol
                            for b in range(BPC):
                                g = ol * BPC + b
                                if not transposed:
                                    nc.vector.tensor_scalar_add(
                                        dst16[:, b, oc, :], pq[g][:],
                                        bias_t[:, oc:oc + 1])
                                else:
                                    vtmp = evpool.tile([128, 512], F32,
                                                       tag="vtmp", bufs=2,
                                                       name="vtmp")
                                    nc.vector.tensor_copy(vtmp[:], pq[g][:])
                                    ptv = psum_tile(g)
                                    for mc in range(4):
                                        nc.tensor.transpose(
                                            ptv[:, mc * 128:(mc + 1) * 128],
                                            vtmp[:, mc * 128:(mc + 1) * 128],
                                            ident[:])
                                    nc.vector.tensor_copy(
                                        vT16[:, b, :,
                                             oc * 128:(oc + 1) * 128],
                                        ptv[:].rearrange("p (m c) -> p m c",
                                                         m=4))

                conv(xpad, wqp, bq_t, False, q16)
                conv(ypad, wkp, bk_t, False, k16)

                # scores + softmax for both batches: emitted before the v
                # conv so the vector/activation work overlaps v's PE time
                attnT = {}
                with tc.tile_pool(name="attn", bufs=1) as dpool:
                    for b in range(BPC):
                        stats = dpool.tile([128, 3, 4], F32, tag=f"stats{b}",
                                           name=f"stats{b}")
                        attn_n = dpool.tile([128, 4, N], F32,
                                            tag=f"attn_n{b}",
                                            name=f"attn_n{b}")
                        for g in range(4):
                            ps = psum_tile(g)
                            for oc in range(ICH):
                                nc.tensor.matmul(
                                    ps[:],
                                    q16[:, b, oc, g * 128:(g + 1) * 128],
                                    k16[:, b, oc, :],
                                    start=(oc == 0), stop=(oc == ICH - 1))
                            negmax = stats[:, 0, g:g + 1]
                            esum = stats[:, 1, g:g + 1]
                            rinv = stats[:, 2, g:g + 1]
                            nc.vector.reduce_max(negmax, ps[:],
                                                 axis=mybir.AxisListType.X,
                                                 negate=True)
                            nc.scalar.activation(
                                attn_n[:, g, :], ps[:],
                                mybir.ActivationFunctionType.Exp,
                                bias=negmax, accum_out=esum)
                            nc.vector.reciprocal(rinv, esum)
                            nc.vector.tensor_scalar_mul(attn_n[:, g, :],
                                                        attn_n[:, g, :],
                                                        rinv)
                        # transpose attn -> attnT16 [m-part, mc, n]
                        aT = dpool.tile([128, 4, N], F16, tag=f"attnT{b}",
                                        name=f"attnT{b}")
                        for mc in range(4):
                            pt = psum_tile(4 + mc)
                            for g in range(4):
                                nc.tensor.transpose(
                                    pt[:, g * 128:(g + 1) * 128],
                                    attn_n[:, g, mc * 128:(mc + 1) * 128],
                                    ident[:])
                            nc.vector.tensor_copy(aT[:, mc, :], pt[:])
                        attnT[b] = aT

                    # ---- v conv: fp8e4 DoubleRow (K=256 per matmul) ----
                    # PSUM accumulates WS*v; descaled during evacuation.
                    for b in range(BPC):
                        pq = [psum_tile(i) for i in range(8)]
                        for t in range(27):
                            for icp in range(4):
                                wt8 = wpool.tile([128, 2, 1024], F8,
                                                 tag="wt8", bufs=4,
                                                 name="wt8")
                                nc.sync.dma_start(wt8[:], wv8[t, icp])
                                yt8 = wpool.tile([128, 2, 512], F8,
                                                 tag="yt8", bufs=4,
                                                 name="yt8")
                                nc.sync.dma_start(yt8[:], y8sl[b, t, icp])
                                first = (t == 0 and icp == 0)
                                last = (t == 26 and icp == 3)
                                for oh in range(2):
                                    for ol in range(4):
                                        o0 = oh * 512 + ol * 128
                                        nc.tensor.matmul(
                                            pq[oh * 4 + ol][:],
                                            wt8[:, :, o0:o0 + 128],
                                            yt8[:],
                                            start=first, stop=last,
                                            perf_mode=mybir.MatmulPerfMode
                                            .DoubleRow)
                        for oh in range(2):
                            for ol in range(4):
                                oc = oh * 4 + ol
                                vtmp = evpool.tile([128, 512], F32,
                                                   tag="vtmp", bufs=2,
                                                   name="vtmp")
                                nc.vector.tensor_scalar_mul(
                                    vtmp[:], pq[oh * 4 + ol][:], 1.0 / WS)
                                ptv = psum_tile(oh * 4 + ol)
                                for mc in range(4):
                                    nc.tensor.transpose(
                                        ptv[:, mc * 128:(mc + 1) * 128],
                                        vtmp[:, mc * 128:(mc + 1) * 128],
                                        ident[:])
                                nc.vector.tensor_copy(
                                    vT16[:, b, :, oc * 128:(oc + 1) * 128],
                                    ptv[:].rearrange("p (m c) -> p m c",
                                                     m=4))

                    # out = vT.T @ attnT + (x + bv)
                    with tc.tile_pool(name="ot", bufs=4) as opool:
                        for b in range(BPC):
                            for oc in range(ICH):
                                xrt = opool.tile([128, N], F32, tag="xrt",
                                                 name="xrt")
                                nc.sync.dma_start(
                                    xrt[:],
                                    xres[b, oc * 128:(oc + 1) * 128, :])
                                po = psum_tile(oc)
                                for mc in range(4):
                                    nc.tensor.matmul(
                                        po[:],
                                        vT16[:, b, mc,
                                             oc * 128:(oc + 1) * 128],
                                        attnT[b][:, mc, :],
                                        start=(mc == 0), stop=(mc == 3))
                                ot = opool.tile([128, N], F32, tag="ot",
                                                name="ot")
                                nc.vector.tensor_add(ot[:], po[:], xrt[:])
                                nc.sync.dma_start(
                                    out[b, oc * 128:(oc + 1) * 128, :],
                                    ot[:])
    nc.compile()
    return nc


def _prep_weight(w):
    # [O, I, kd, kh, kw] -> [ICH, oh, tg, p, t9, o'] fp16 so each (ic, oh,
    # tg) block is one fully-contiguous 9KB-per-partition DMA
    w27 = w.transpose(2, 3, 4, 1, 0).reshape(27, C, C)  # [t, i, o]
    w_re = w27.reshape(3, 9, ICH, 128, 2, 512).transpose(2, 4, 0, 3, 1, 5)
    return np.ascontiguousarray(w_re).astype(np.float16)


def kernel(x, y, wq, bq, wk, bk, wv, bv):
    global _CACHED_NC, LAST_RESULTS
    x = np.asarray(x, np.float32)
    y = np.asarray(y, np.float32)

    E4 = ml_dtypes.float8_e4m3
    xf = x.reshape(B, C, 8, 8, 8)
    yf = y.reshape(B, C, 8, 8, 8)
    xpad = np.zeros((B, C, 10, 10, 10), np.float16)
    xpad[:, :, 1:9, 1:9, 1:9] = xf
    ypad32 = np.zeros((B, C, 10, 10, 10), np.float32)
    ypad32[:, :, 1:9, 1:9, 1:9] = yf
    xpad = xpad.reshape(B, C, PAD)
    ypad = ypad32.astype(np.float16).reshape(B, C, PAD)
    xres = x.reshape(B, C, N) + np.asarray(bv, np.float32)[None, :, None]

    # fp8 v-conv operands: e4m3 activations as per-tap flat slabs
    # [b, t, icp, p, j, n], e4m3 weights (x WS) as [t, icp, p, j, o]
    y8 = ypad32.astype(E4)
    sw = np.lib.stride_tricks.sliding_window_view(
        y8, (8, 8, 8), axis=(2, 3, 4))          # [B, C, 3,3,3, 8,8,8]
    arr = np.ascontiguousarray(sw.transpose(0, 2, 3, 4, 1, 5, 6, 7))
    y8sl = np.ascontiguousarray(
        arr.reshape(B, 27, 4, 2, 128, 512).transpose(0, 1, 2, 4, 3, 5))

    w27v = np.asarray(wv, np.float32).transpose(2, 3, 4, 1, 0).reshape(
        27, C, C)
    w8 = (w27v * WS).astype(E4)
    wv8 = np.ascontiguousarray(
        w8.reshape(27, 4, 2, 128, C).transpose(0, 1, 3, 2, 4))

    wqp = _prep_weight(np.asarray(wq, np.float32))
    wkp = _prep_weight(np.asarray(wk, np.float32))
    bqp = np.ascontiguousarray(
        np.asarray(bq, np.float32).reshape(ICH, 128).T)
    bkp = np.ascontiguousarray(
        np.asarray(bk, np.float32).reshape(ICH, 128).T)

    if _CACHED_NC is None:
        _CACHED_NC = _build()

    in_maps = []
    for i in range(NCORES):
        s = slice(i * BPC, (i + 1) * BPC)
        in_maps.append({
            "xp": xpad[s], "yp": ypad[s], "xres": xres[s],
            "wqp": wqp, "wkp": wkp, "wv8": wv8, "y8sl": y8sl[s],
            "bqp": bqp, "bkp": bkp,
        })

    res = run_bass_kernel_spmd(_CACHED_NC, in_maps, list(range(NCORES)))
    LAST_RESULTS = res
    full = np.concatenate([res.results[i]["out"] for i in range(NCORES)],
                          axis=0)
    return full.reshape(B, C, 8, 8, 8)
